# revision 23
# baseline (speedup 1.0000x reference)
"""Causal single-head attention (B=4, S=2048, d=1024) on 8 TRN2 NeuronCores.

Sharding (uniform single program): core c -> batch b = c//2, subset
s = c%2. Per batch, the 16 query blocks of 128 rows are split into
quads t=0..3; core (b,s) owns blocks {4t+2s, 4t+2s+1}. Every core runs
the identical instruction stream (padded causal limit (t+1)*512 per
quad); the true causal boundary comes from per-core 0/1 mask tiles
supplied as input data.

K/V projections are tensor-parallel within each core pair: core (b,s)
computes the d_out-half s of kT and v for the whole batch; halves are
exchanged with a pairwise AllGather ([[0,1],[2,3],[4,5],[6,7]]).

All device inputs are host-preswizzled to partition-major [128, ...]
layouts so every DMA moves large contiguous per-partition segments.

Compute (bf16 operands, fp32 PSUM accumulate):
  P1: kT half -> kg_in, AllGather -> kt [d_out, 2048]
  P2: v half  -> vg_in, AllGather -> vv [2048, d_out]
  P0: qT = (Wq/sqrt(d)) @ xq^T -> resident SBUF (overlaps the gathers)
  P3: per quad t: scoresT[k,q] = kt.T-slices @ qT-group, p = exp(scoresT)
      (no max subtraction: |scores| <= ~2), mask, then
      out[q,:] = (pT.T @ v) / (pT.T @ ones)  -- row sums via ones-matmul.
"""
import sys

sys.path.insert(0, "/opt/trn_rl_repo")

import ml_dtypes
import numpy as np

import concourse.bass as bass  # noqa: F401
import concourse.mybir as mybir
import concourse.tile as tile
from concourse import bacc
from concourse.bass_utils import run_bass_kernel_spmd

B, S, D = 4, 2048, 1024
DC = D // 128          # 8 contraction chunks
NKB = S // 128         # 16 key blocks
SCALE = 1.0 / float(np.sqrt(D))
F32 = mybir.dt.float32
BF = mybir.dt.bfloat16
EXP = mybir.ActivationFunctionType.Exp
GROUPS = [[0, 1], [2, 3], [4, 5], [6, 7]]

_cache = {}


def build_nc():
    nc = bacc.Bacc("TRN2", target_bir_lowering=False, debug=False, num_devices=8)
    # all inputs partition-major: [128, dc, cols]
    xT = nc.dram_tensor("xT", [128, DC, S], BF, kind="ExternalInput")
    xTq = nc.dram_tensor("xTq", [128, DC, 1024], BF, kind="ExternalInput")
    WqT = nc.dram_tensor("WqT", [128, DC, D], BF, kind="ExternalInput")
    WkTh = nc.dram_tensor("WkTh", [128, DC, 512], BF, kind="ExternalInput")
    WvTh = nc.dram_tensor("WvTh", [128, DC, 512], BF, kind="ExternalInput")
    masks = nc.dram_tensor("masks", [128, 4, 256], BF, kind="ExternalInput")
    out = nc.dram_tensor("out", [1024, D], F32, kind="ExternalOutput")
    # collective buffers, partition-major SBUF images
    kg_in = nc.dram_tensor("kg_in", [128, 4, S], BF)
    kg_out = nc.dram_tensor("kg_out", [2, 128, 4, S], BF)
    vg_in = nc.dram_tensor("vg_in", [128, NKB, 512], BF)
    vg_out = nc.dram_tensor("vg_out", [2, 128, NKB, 512], BF)

    with tile.TileContext(nc) as tc:
        with (
            tc.tile_pool(name="w", bufs=1) as wp,
            tc.tile_pool(name="per", bufs=1) as per,
            tc.tile_pool(name="px", bufs=2) as pxp,
            tc.tile_pool(name="ev", bufs=6) as evp,
            tc.tile_pool(name="pt", bufs=42) as ptp,
            tc.tile_pool(name="ot", bufs=1) as otp,
            tc.tile_pool(name="sml", bufs=4) as smlp,
            tc.tile_pool(name="mix", bufs=5, space="PSUM") as mixp,
            tc.tile_pool(name="psav", bufs=3, space="PSUM") as psavp,
        ):
            # ---------------- consts + persistent ----------------
            kt = per.tile([128, DC, S], BF)        # kT: [d_out, 2048]
            vv = per.tile([128, 2, NKB, 512], BF)  # v: [2048, (rank0|rank1) 512]
            qt = per.tile([128, DC, 1024], BF)     # qT: [d_out, 1024]
            kg_sb = per.tile([128, 4, S], BF)      # K-half staging
            vg_sb = per.tile([128, NKB, 512], BF)  # V-half staging
            zeros_f = per.tile([128, 2], F32)
            ones = per.tile([128, 2], BF)
            maskt = per.tile([128, 4, 256], BF)
            nc.vector.memset(zeros_f, 0.0)
            # exp(0)=1 -> also preloads the ACT exp table long before P3
            nc.scalar.activation(ones, zeros_f, EXP)
            nc.sync.dma_start(out=maskt, in_=masks[:])

            # -------- P1: K half-projection -> kg_in --------
            wk = wp.tile([128, DC, 512], BF)
            nc.sync.dma_start(out=wk, in_=WkTh[:])
            for sc in range(4):
                xk = pxp.tile([128, DC, 512], BF, tag="xs", name=f"xk_{sc}")
                nc.sync.dma_start(out=xk, in_=xT[:, :, sc * 512:(sc + 1) * 512])
                for ocl in range(4):
                    ps = mixp.tile([128, 512], F32, tag="mix")
                    for dc in range(DC):
                        nc.tensor.matmul(
                            ps,
                            lhsT=wk[:, dc, ocl * 128:(ocl + 1) * 128],
                            rhs=xk[:, dc, :],
                            start=(dc == 0),
                            stop=(dc == DC - 1),
                        )
                    nc.vector.tensor_copy(
                        kg_sb[:, ocl, sc * 512:(sc + 1) * 512], ps
                    )

            nc.scalar.dma_start(out=kg_in[:], in_=kg_sb)
            # -------- AllGather K halves (overlaps P2/P0) --------
            nc.gpsimd.collective_compute(
                "AllGather",
                mybir.AluOpType.bypass,
                replica_groups=GROUPS,
                ins=[kg_in[:]],
                outs=[kg_out[:]],
            )

            # -------- P2: V half-projection -> vg_in --------
            wv = wp.tile([128, DC, 512], BF)
            nc.sync.dma_start(out=wv, in_=WvTh[:])
            for sc in range(4):
                xv = pxp.tile([128, DC, 512], BF, tag="xs", name=f"xv_{sc}")
                nc.sync.dma_start(out=xv, in_=xT[:, :, sc * 512:(sc + 1) * 512])
                for sb in range(4):
                    ps = mixp.tile([128, 512], F32, tag="mix", name=f"ps2_{sc}_{sb}")
                    for dc in range(DC):
                        nc.tensor.matmul(
                            ps,
                            lhsT=xv[:, dc, sb * 128:(sb + 1) * 128],
                            rhs=wv[:, dc, :],
                            start=(dc == 0),
                            stop=(dc == DC - 1),
                        )
                    nc.vector.tensor_copy(vg_sb[:, sc * 4 + sb, :], ps)

            nc.scalar.dma_start(out=vg_in[:], in_=vg_sb)
            # -------- AllGather V halves (overlaps P0) --------
            nc.gpsimd.collective_compute(
                "AllGather",
                mybir.AluOpType.bypass,
                replica_groups=GROUPS,
                ins=[vg_in[:]],
                outs=[vg_out[:]],
            )

            # -------- P0: Q projection -> qt (overlaps the gathers) --------
            wq = wp.tile([128, DC, D], BF)
            xq = wp.tile([128, DC, 1024], BF)
            nc.sync.dma_start(out=wq, in_=WqT[:])
            nc.sync.dma_start(out=xq, in_=xTq[:])
            for oc in range(8):
                pss = [
                    mixp.tile([128, 512], F32, tag="mix", name=f"ps0_{oc}_{i}")
                    for i in range(2)
                ]
                for dc in range(DC):
                    for sc in range(2):
                        nc.tensor.matmul(
                            pss[sc],
                            lhsT=wq[:, dc, oc * 128:(oc + 1) * 128],
                            rhs=xq[:, dc, sc * 512:(sc + 1) * 512],
                            start=(dc == 0),
                            stop=(dc == DC - 1),
                        )
                for sc in range(2):
                    nc.vector.tensor_copy(
                        qt[:, oc, sc * 512:(sc + 1) * 512], pss[sc]
                    )

            # -------- load gathered kt / vv (2 big DMAs each) --------
            for ch in range(2):
                for r in range(2):
                    nc.sync.dma_start(
                        out=kt[:, r * 4:(r + 1) * 4, ch * 1024:(ch + 1) * 1024],
                        in_=kg_out[r][:, :, ch * 1024:(ch + 1) * 1024],
                    )
            for r in range(2):
                nc.sync.dma_start(out=vv[:, r, :, :], in_=vg_out[r])

            # ---------------- P3: attention ----------------
            # Phase A: all scoresT + exp + mask (needs kt/qt only)
            all_pts = {}
            for t in range(4):
                L = 4 * t + 4
                for kb in range(L):
                    ps = mixp.tile([128, 512], F32, tag="mix")
                    for dc in range(DC):
                        nc.tensor.matmul(
                            ps[:, 0:256],
                            lhsT=kt[:, dc, kb * 128:(kb + 1) * 128],
                            rhs=qt[:, dc, t * 256:(t + 1) * 256],
                            start=(dc == 0),
                            stop=(dc == DC - 1),
                        )
                    pt = ptp.tile([128, 256], BF, tag="pt")
                    nc.scalar.activation(pt, ps[:, 0:256], EXP)
                    kbr = kb - 4 * t
                    if kbr >= 0:
                        nc.vector.tensor_mul(pt, pt, maskt[:, kbr, :])
                    all_pts[(t, kb)] = pt
            # Phase B: all l + av (needs vv from the second gather)
            for t in range(4):
                L = 4 * t + 4
                for j in range(2):
                    qsl = slice(j * 128, (j + 1) * 128)
                    lps = psavp.tile([128, 2], F32, tag="psav", name=f"l_{t}_{j}")
                    for kb in range(L):
                        nc.tensor.matmul(
                            lps,
                            lhsT=all_pts[(t, kb)][:, qsl],
                            rhs=ones,
                            start=(kb == 0),
                            stop=(kb == L - 1),
                        )
                    rec = smlp.tile([128, 1], F32, tag="rec")
                    nc.vector.reciprocal(rec, lps[:, 0:1])
                    ot = otp.tile([128, D], F32, tag="ot")
                    for oh in range(2):
                        avp = psavp.tile([128, 512], F32, tag="psav",
                                         name=f"av_{t}_{j}_{oh}")
                        for kb in range(L):
                            nc.tensor.matmul(
                                avp,
                                lhsT=all_pts[(t, kb)][:, qsl],
                                rhs=vv[:, oh, kb, :],
                                start=(kb == 0),
                                stop=(kb == L - 1),
                            )
                        nc.vector.tensor_scalar_mul(
                            ot[:, oh * 512:(oh + 1) * 512], avp, rec
                        )
                    nc.scalar.dma_start(
                        out=out[t * 256 + j * 128: t * 256 + (j + 1) * 128, :],
                        in_=ot,
                    )
    nc.compile()
    return nc


def _query_cols(sub):
    return np.concatenate(
        [
            np.arange((4 * t + 2 * sub) * 128, (4 * t + 2 * sub + 2) * 128)
            for t in range(4)
        ]
    )


def _masks(sub):
    m = np.zeros((4, 128, 256), np.float32)
    p = np.arange(128)[:, None]
    j = np.arange(256)[None, :]
    qoff = (2 * sub + j // 128) * 128 + j % 128
    for kbr in range(4):
        m[kbr] = (kbr * 128 + p <= qoff).astype(np.float32)
    return np.ascontiguousarray(m.transpose(1, 0, 2))  # -> [128, 4, 256]


def _pmaj(a):
    """[dc*128, cols] -> partition-major [128, dc, cols]."""
    d, cols = a.shape
    return np.ascontiguousarray(a.reshape(d // 128, 128, cols).transpose(1, 0, 2))


def kernel(x, Wq, Wk, Wv, _trace=False):
    if "nc" not in _cache:
        _cache["nc"] = build_nc()
    nc = _cache["nc"]

    bf = ml_dtypes.bfloat16
    x = np.asarray(x, dtype=np.float32)
    WqT = _pmaj((np.asarray(Wq, np.float32).T * np.float32(SCALE)).astype(bf))
    WkT = np.asarray(Wk, np.float32).T.astype(bf)
    WvT = np.asarray(Wv, np.float32).T.astype(bf)

    in_maps = []
    for c in range(8):
        b, sub = c // 2, c % 2
        xT = x[b].T.astype(bf)
        in_maps.append(
            {
                "xT": _pmaj(xT),
                "xTq": _pmaj(np.ascontiguousarray(xT[:, _query_cols(sub)])),
                "WqT": WqT,
                "WkTh": _pmaj(WkT[:, sub * 512:(sub + 1) * 512]),
                "WvTh": _pmaj(WvT[:, sub * 512:(sub + 1) * 512]),
                "masks": _masks(sub).astype(bf),
            }
        )

    res = run_bass_kernel_spmd(
        nc, in_maps, core_ids=list(range(8)), trace=_trace
    )
    full = np.empty((B, S, D), np.float32)
    for c in range(8):
        b, sub = c // 2, c % 2
        full[b, _query_cols(sub)] = res.results[c]["out"]
    if _trace:
        _cache["last_result"] = res
    return full
